# revision 1
# baseline (speedup 1.0000x reference)
"""Trainium2 Bass kernel for nn_ColumnUniform (GNN message passing).

Computes, for a graph with N nodes and E edges (edge_index = [row; col]):
    rowsum[n] = sum of edge_attr over edges with row index n
    out[e]    = edge_attr[e] / rowsum[col[e]]

Strategy (8 NeuronCores, SPMD, fully streaming device kernel):
  Sharding: node range c = [c*N/8, (c+1)*N/8). Core c receives
    - phase A: all edges whose ROW is in range c (for the row sums), and
    - phase B: all edges whose COL is in range c (for the scaling),
  so the per-range reciprocal table is produced and consumed on the same
  core: no inter-core communication at all.

  Layout (host, pure index manipulation): nodes of a range are grouped
  into (col-degree-class, row-degree-class) cells; each node gets a
  fixed-size slot of D cells in the phase-A stream (its row-edges,
  zero-padded) and of Ec cells in the phase-B stream (its col-edges).
  Degree classes are chosen from the data by a small DP so padding is a
  few percent.

  Device: phase A = windowed tensor_reduce (window D) over the resident
  phase-A stream -> rowsum per slot -> reciprocal (+1 Newton step), all
  kept in SBUF. Phase B = one broadcast-multiply per tile: each slot's
  reciprocal value times its Ec col-edge attrs, streamed out. The output
  leaves the device in slot order; the host inverts the (pure
  permutation) layout when unsharding.
"""
import sys

for _p in ("/opt/trn_rl_repo", "/root/.axon_site/_ro/trn_rl_repo"):
    if _p not in sys.path:
        sys.path.append(_p)

import numpy as np

import concourse.bass as bass
import concourse.mybir as mybir
from concourse.bass_utils import run_bass_kernel_spmd

DT = mybir.dt.float32

N_CORES = 8
P = 128               # SBUF partitions
ACHUNK = 2048         # phase-A load chunk (free-dim columns)
BCHUNK = 4096         # phase-B tile width budget (free-dim columns)
NCLASS = 12           # degree classes per side


# ----------------------------------------------------------------------------
# Host-side layout (the sharding strategy): integer index work only.
# ----------------------------------------------------------------------------

def _pick_classes(deg, K):
    """Choose <=K class ceilings for positive degrees minimizing total slot
    cells (DP over quantile candidates)."""
    d = deg[deg > 0]
    dmax = int(d.max())
    cnt = np.bincount(d, minlength=dmax + 1)
    csum = np.concatenate([[0], np.cumsum(cnt)])
    cand = np.unique(np.concatenate([
        np.quantile(d, np.linspace(0, 1, K * 4)).astype(np.int64), [dmax]]))
    cand = cand[cand > 0]
    M = len(cand)
    INF = float("inf")
    dp = np.full((K + 1, M), INF)
    par = np.zeros((K + 1, M), np.int64)
    for j in range(M):
        dp[1][j] = csum[cand[j] + 1] * cand[j]
    for k in range(2, K + 1):
        for j in range(1, M):
            pj = np.arange(j)
            costs = dp[k - 1][pj] + (csum[cand[j] + 1] - csum[cand[pj] + 1]) * cand[j]
            i = int(np.argmin(costs))
            dp[k][j] = costs[i]
            par[k][j] = pj[i]
    ks = int(np.argmin(dp[:, M - 1]))
    out = []
    k, j = ks, M - 1
    while k >= 1:
        out.append(int(cand[j]))
        j = int(par[k][j])
        k -= 1
    return np.array(sorted(out), np.int64)


def prepare(edge_index, edge_attr, n_nodes):
    row = np.asarray(edge_index[0]).astype(np.int64)
    col = np.asarray(edge_index[1]).astype(np.int64)
    attr = np.asarray(edge_attr, dtype=np.float32)
    E = attr.shape[0]
    N = int(n_nodes)
    NR = (N + N_CORES - 1) // N_CORES

    rd = np.bincount(row, minlength=N)
    cd = np.bincount(col, minlength=N)
    clD = _pick_classes(rd, NCLASS)
    clE = _pick_classes(cd, NCLASS)
    Dn = clD[np.searchsorted(clD, np.maximum(rd, 1))]
    En = np.where(cd > 0, clE[np.searchsorted(clE, np.maximum(cd, 1))], 0)

    core = np.minimum(np.arange(N) // NR, N_CORES - 1)
    # cell id = (E-class index + 1 [0 for cd==0], D-class index), E-major
    eidx = np.where(cd > 0, np.searchsorted(clE, np.maximum(cd, 1)) + 1, 0)
    didx = np.searchsorted(clD, np.maximum(rd, 1))
    cellkey = eidx * (len(clD) + 1) + didx

    # node order within (core, cell); nodes ascending keeps things stable
    order = np.lexsort((np.arange(N), cellkey, core))
    oc = core[order]
    ock = cellkey[order]
    # group start positions
    grp = oc * (cellkey.max() + 1) + ock
    starts = np.concatenate([[0], np.nonzero(np.diff(grp))[0] + 1])
    gstart = np.zeros(N, np.int64)
    gstart[starts] = starts
    np.maximum.accumulate(gstart, out=gstart)
    krank = np.arange(N) - gstart          # slot index k within (core, cell)
    kn = np.empty(N, np.int64)
    kn[order] = krank

    # per-cell max count over cores -> shared geometry
    ucell, uinv = np.unique(cellkey, return_inverse=True)
    CNC = len(ucell)
    counts = np.zeros((N_CORES, CNC), np.int64)
    np.add.at(counts, (core, uinv), 1)
    smax = counts.max(axis=0)
    ka = -(-smax // P)                      # column groups per cell
    cellD = clD[ucell % (len(clD) + 1)]
    cellE = np.where(ucell >= (len(clD) + 1),
                     clE[np.maximum(ucell // (len(clD) + 1) - 1, 0)], 0)

    wa_w = ka * cellD
    wb_w = ka * cellE                       # 0 for the cd==0 cells
    ca = np.concatenate([[0], np.cumsum(wa_w)])[:-1]
    cv = np.concatenate([[0], np.cumsum(ka)])[:-1]
    cb = np.concatenate([[0], np.cumsum(wb_w)])[:-1]
    WA = int(wa_w.sum())
    WV = int(ka.sum())
    WB = int(wb_w.sum())

    # per-node placement
    ci = uinv                               # cell index per node
    pn = kn % P
    jn = kn // P
    acol0 = ca[ci] + jn * cellD[ci]
    bcol0 = cb[ci] + jn * cellE[ci]

    # ranks of edges within row / col
    def ranks(keys):
        ptr = np.zeros(N + 1, np.int64)
        np.cumsum(np.bincount(keys, minlength=N), out=ptr[1:])
        prm = np.argsort(keys, kind="stable")
        r = np.arange(E, dtype=np.int64) - ptr[keys[prm]]
        out = np.empty(E, np.int64)
        out[prm] = r
        return out

    rrank = ranks(row)
    crank = ranks(col)

    # scatter attr into per-core A and B streams
    attr_a = np.zeros((N_CORES, P, WA), np.float32)
    attr_b = np.zeros((N_CORES, P, WB), np.float32)
    fa = core[row] * (P * WA) + pn[row] * WA + acol0[row] + rrank
    attr_a.reshape(-1)[fa] = attr
    fb = core[col] * (P * WB) + pn[col] * WB + bcol0[col] + crank
    attr_b.reshape(-1)[fb] = attr

    in_maps = [{"attr_a": attr_a[c], "attr_b": attr_b[c]} for c in range(N_CORES)]

    # cells in stream order for the device program
    cells = []
    for i in range(CNC):
        cells.append(dict(D=int(cellD[i]), E=int(cellE[i]), ka=int(ka[i]),
                          ca=int(ca[i]), cv=int(cv[i]), cb=int(cb[i])))
    geom = dict(WA=WA, WB=WB, WV=WV, cells=cells)
    # info to unshard: position of each edge in the B stream of its core
    fb_local = pn[col] * WB + bcol0[col] + crank
    return in_maps, geom, (core[col], fb_local)


def unshard(results, E, geom, binfo):
    bcore, fb_local = binfo
    outs = np.stack([results[c]["out"].reshape(-1) for c in range(N_CORES)])
    return outs[bcore, fb_local]


# ----------------------------------------------------------------------------
# Device program
# ----------------------------------------------------------------------------

def build_program(geom, debug=False):
    WA = geom["WA"]
    WB = geom["WB"]
    WV = geom["WV"]
    cells = geom["cells"]

    nc = bass.Bass()
    attr_a = nc.declare_dram_parameter("attr_a", [P, WA], DT, isOutput=False)
    attr_b = nc.declare_dram_parameter("attr_b", [P, WB], DT, isOutput=False)
    out_ext = nc.declare_dram_parameter("out", [P, WB], DT, isOutput=True)
    if debug:
        v_dbg = nc.declare_dram_parameter("v_dbg", [P, WV], DT, isOutput=True)
        rs_dbg = nc.declare_dram_parameter("rs_dbg", [P, WV], DT, isOutput=True)
        a_dbg = nc.declare_dram_parameter("a_dbg", [P, WA], DT, isOutput=True)

    # phase-A load chunks
    na = (WA + ACHUNK - 1) // ACHUNK
    # last chunk needed by each cell
    cell_chunk = [((c["ca"] + c["ka"] * c["D"] - 1) // ACHUNK) for c in cells]

    # phase-B groups: one load+store of <=BCHUNK contiguous columns, with
    # per-cell broadcast-multiply segments inside (split at slot boundaries)
    groups = []  # (g0, width, [(off, vcol0, k, e), ...])
    cur0 = None
    cur_w = 0
    cur_segs = []
    for c in cells:
        e = c["E"]
        if e == 0:
            continue
        k0 = 0
        while k0 < c["ka"]:
            if cur0 is None:
                cur0, cur_w, cur_segs = c["cb"] + k0 * e, 0, []
            room = (BCHUNK - cur_w) // e
            if room == 0:
                groups.append((cur0, cur_w, cur_segs))
                cur0, cur_w, cur_segs = c["cb"] + k0 * e, 0, []
                room = BCHUNK // e
            k = min(room, c["ka"] - k0)
            if (cur_segs and cur_segs[-1][3] == e
                    and cur_segs[-1][0] + cur_segs[-1][2] * e == cur_w
                    and cur_segs[-1][1] + cur_segs[-1][2] == c["cv"] + k0):
                off, v0, pk, _ = cur_segs[-1]
                cur_segs[-1] = (off, v0, pk + k, e)
            else:
                cur_segs.append((cur_w, c["cv"] + k0, k, e))
            cur_w += k * e
            k0 += k
    if cur0 is not None and cur_w:
        groups.append((cur0, cur_w, cur_segs))
    NG = len(groups)
    NBUF = 4

    from contextlib import ExitStack
    with ExitStack() as ctx:
        block = ctx.enter_context(nc.Block())
        sA = [ctx.enter_context(nc.semaphore(f"sA{i}")) for i in range(na)]
        sRed = ctx.enter_context(nc.semaphore("sRed"))   # cells reduced
        sV = ctx.enter_context(nc.semaphore("sV"))       # v table ready
        sBin = [ctx.enter_context(nc.semaphore(f"sBin{i}")) for i in range(NBUF)]
        sMul = ctx.enter_context(nc.semaphore("sMul"))   # B tiles multiplied
        sBout = [ctx.enter_context(nc.semaphore(f"sBout{i}")) for i in range(NBUF)]
        sDbg = ctx.enter_context(nc.semaphore("sDbg"))

        A_sb = ctx.enter_context(nc.sbuf_tensor("A_sb", [P, WA], DT))
        v_sb = ctx.enter_context(nc.sbuf_tensor("v_sb", [P, WV], DT))
        t_sb = ctx.enter_context(nc.sbuf_tensor("t_sb", [P, WV], DT))
        bt = [ctx.enter_context(nc.sbuf_tensor(f"bt{i}", [P, BCHUNK], DT))
              for i in range(NBUF)]

        @block.sync
        def _(sync):
            for i in range(na):
                w0 = i * ACHUNK
                w1 = min(WA, w0 + ACHUNK)
                sync.dma_start(out=A_sb[:, w0:w1], in_=attr_a[:, w0:w1]).then_inc(sA[i], 16)
            for g, (g0, w, segs) in enumerate(groups):
                if g >= NBUF:
                    # same buffer's previous store completed
                    sync.wait_ge(sBout[g % NBUF], 16 * ((g - NBUF) // NBUF + 1))
                sync.dma_start(out=bt[g % NBUF][:, :w], in_=attr_b[:, g0:g0 + w]).then_inc(sBin[g % NBUF], 16)

        @block.vector
        def _(vector):
            for i, c in enumerate(cells):
                c0 = c["ca"] // ACHUNK
                for ch in range(c0, cell_chunk[i] + 1):
                    vector.wait_ge(sA[ch], 16)
                ka, D, ca, cv = c["ka"], c["D"], c["ca"], c["cv"]
                src = A_sb[:, ca:ca + ka * D].rearrange("p (k d) -> p k d", d=D)
                vector.tensor_reduce(
                    out=v_sb[:, cv:cv + ka], in_=src,
                    axis=mybir.AxisListType.X, op=mybir.AluOpType.add,
                ).then_inc(sRed, 1)
            # v = 1/rowsum with one Newton refinement
            vector.wait_ge(sRed, len(cells))
            if debug:
                vector.wait_ge(sDbg, 16)
            vector.reciprocal(t_sb[:, :], v_sb[:, :])
            vector.tensor_mul(v_sb[:, :], v_sb[:, :], t_sb[:, :])      # x*r
            vector.tensor_scalar(out=v_sb[:, :], in0=v_sb[:, :],
                                 scalar1=-1.0, scalar2=2.0,
                                 op0=mybir.AluOpType.mult, op1=mybir.AluOpType.add)
            vector.tensor_mul(v_sb[:, :], v_sb[:, :], t_sb[:, :]).then_inc(sV, 1)
            # phase-B broadcast multiplies (in place on the loaded tile)
            vector.wait_ge(sV, 1)
            for g, (g0, w, segs) in enumerate(groups):
                vector.wait_ge(sBin[g % NBUF], 16 * (g // NBUF + 1))
                for si, (off, v0, k, e) in enumerate(segs):
                    dst = bt[g % NBUF][:, off:off + k * e].rearrange(
                        "p (k e) -> p k e", e=e)
                    inst = vector.tensor_tensor(
                        out=dst, in0=v_sb[:, v0:v0 + k, None].to_broadcast([P, k, e]),
                        in1=dst, op=mybir.AluOpType.mult,
                    )
                    if si == len(segs) - 1:
                        inst.then_inc(sMul, 1)

        @block.scalar
        def _(scalar):
            if debug:
                scalar.wait_ge(sRed, len(cells))
                scalar.dma_start(out=rs_dbg[:, :], in_=v_sb[:, :]).then_inc(sDbg, 16)
                scalar.wait_ge(sV, 1)
                scalar.dma_start(out=v_dbg[:, :], in_=v_sb[:, :]).then_inc(sDbg, 16)
                scalar.dma_start(out=a_dbg[:, :], in_=A_sb[:, :]).then_inc(sDbg, 16)
                scalar.wait_ge(sDbg, 48)
            for g, (g0, w, segs) in enumerate(groups):
                scalar.wait_ge(sMul, g + 1)
                scalar.dma_start(out=out_ext[:, g0:g0 + w], in_=bt[g % NBUF][:, :w]).then_inc(sBout[g % NBUF], 16)

    return nc


# ----------------------------------------------------------------------------
# Entry point
# ----------------------------------------------------------------------------

def kernel(edge_index, edge_attr, N):
    import os
    edge_index = np.asarray(edge_index)
    edge_attr = np.asarray(edge_attr)
    E = edge_attr.shape[0]
    in_maps, geom, binfo = prepare(edge_index, edge_attr, int(N))
    nc = build_program(geom, debug=os.environ.get("KDBG") not in (None, "", "0"))
    trace = os.environ.get("KTRACE") not in (None, "", "0")
    if trace:
        import types
        import antenv
        if "antenv.axon_hooks" not in sys.modules:
            mod = types.ModuleType("antenv.axon_hooks")
            _h = [None]
            mod.set_axon_ntff_profile_hook = lambda h: _h.__setitem__(0, h)
            mod.get_axon_ntff_profile_hook = lambda: _h[0]
            sys.modules["antenv.axon_hooks"] = mod
            antenv.axon_hooks = mod
            from trn_agent_boot.trn_boot import _ntff_profile_via_ctypes
            mod.set_axon_ntff_profile_hook(
                _ntff_profile_via_ctypes("/opt/axon/libaxon_pjrt.so"))
    res = run_bass_kernel_spmd(nc, in_maps, list(range(N_CORES)), trace=trace)
    kernel.last = (res, in_maps, geom)
    return unshard(res.results, E, geom, binfo)


if __name__ == "__main__":
    rng = np.random.default_rng(0)
    N = 4096
    E = 65536
    row = np.concatenate([np.arange(N, dtype=np.int32),
                          rng.integers(0, N, E - N, dtype=np.int32)])
    col = rng.integers(0, N, E, dtype=np.int32)
    attr = rng.random(E, dtype=np.float32) * 0.9 + 0.1
    out = kernel(np.stack([row, col]), attr, N)
    rowsum = np.zeros(N, np.float64)
    np.add.at(rowsum, row, attr.astype(np.float64))
    exp = (1.0 / rowsum)[col] * attr
    err = np.abs(out - exp) / np.abs(exp)
    print("max rel err:", err.max())

